# revision 15
# baseline (speedup 1.0000x reference)
"""Trainium2 Bass kernel for nn_Connector_77738908057780 (dense_mlp).

Computation (see reference):
  x   = image_features                      [B, N, H]    bf16
  f1  = mean(hidden[0:13],  axis=0)         [B, N, H]
  f2  = mean(hidden[13:26], axis=0)         [B, N, H]
  cat = concat([x, f1, f2], -1)             [B, N, 3H]
  h   = gelu(cat @ W1.T + b1)               W1 = nf4_dequant(codes1, scales1) [H, 3H]
  fg  = h @ W2.T + b2                       W2 = nf4_dequant(codes2, scales2) [H, H]
  out = w * LN(fg) + (1-w) * LN(x),         w = sigmoid(alpha)

Sharding: data-parallel over batch B=8 -> one batch element per NeuronCore.

Per-core schedule (v2, software-pipelined over 3 token supertiles
(256, 256, 217) with no overlap):
  - sync HWDGE queue carries the dependency-free streaming loads in time
    order (x tile, 26 hidden layers per supertile) plus the xbar
    transposes that build cat^T; the scalar HWDGE queue carries weights/
    consts up front, then GELU/sqrt activations and output stores.
  - 26-layer sums on DVE add-chains tracking the DMA stream; cat^T via
    SBUF->SBUF xbar transpose; GEMM1 weights-stationary producing h^T in
    PSUM; GELU(+b1) on ACT -> g^T feeds GEMM2 as stationary, producing
    fg token-major.
  - LN stats via DVE bn_stats/bn_aggr; rsqrt via DVE reciprocal + ACT
    sqrt; normalize+gate combine split DVE/GPSIMD (GPSIMD has no PSUM
    port, so PSUM-evacuating ops stay on DVE).
  - Emission order software-pipelines the engines: tail LN/combine work
    of supertile s is emitted between the two load halves of supertile
    s+1 so no engine FIFO head-blocks the DMA stream.

The last supertile has 217 tokens: its second 128-row subtile holds 89
real tokens + 39 garbage rows that flow through compute (finite or NaN)
and are simply never stored.

NF4 dequant of the (small, replicated) weights is host-side weight prep;
the bf16 weights are less DMA traffic than the int32 codes.
"""

import os
import sys

import numpy as np
import ml_dtypes

for _p in ("/opt/trn_rl_repo", "/root/.axon_site/_ro/trn_rl_repo"):
    if os.path.isdir(_p) and _p not in sys.path:
        sys.path.insert(0, _p)

import concourse.bass as bass
import concourse.mybir as mybir
import concourse.tile as tile
from concourse import bacc
from concourse import bass_utils
from concourse import masks

BF16 = mybir.dt.bfloat16
F32 = mybir.dt.float32
AF = mybir.ActivationFunctionType
ALU = mybir.AluOpType

NP_BF16 = ml_dtypes.bfloat16

P = 128
H = 1152
H3 = 3456
NT = 729          # tokens per core (N); B=8 cores
L = 26
KO1 = H3 // P     # 27 k-tiles for GEMM1
KO2 = H // P      # 9 k-tiles for GEMM2
MO = H // P       # 9 output-feature tiles
EPS = 1e-5
NCHUNK = 3        # fg free-dim chunks of 384
CH = H // NCHUNK  # 384

# Supertiles of exactly 256 tokens; the last overlaps the previous by 39
# tokens (473..511 computed twice, identical values stored twice) so that
# every DMA/compute tile is a full 128-partition tile (729 is not a
# multiple of 128; partial tiles would leave stale SBUF rows flowing into
# the transposes and stats ops).
SUPS = [(0, 256), (256, 256), (473, 256)]
NSUB = 2

NF4_CODEBOOK = np.array([
    -1.0, -0.6961928009986877, -0.5250730514526367, -0.39491748809814453,
    -0.28444138169288635, -0.18477343022823334, -0.09105003625154495, 0.0,
    0.07958029955625534, 0.16093020141124725, 0.24611230194568634,
    0.33791524171829224, 0.4407098591327667, 0.5626170039176941,
    0.7229568362236023, 1.0], dtype=np.float32)

BLOCK = 64


def _dequant_nf4(codes, scales):
    """Match reference: codebook lookup * per-64-block absmax, cast bf16."""
    out_f, in_f = codes.shape
    w = NF4_CODEBOOK[codes].reshape(out_f, in_f // BLOCK, BLOCK)
    w = w * scales[:, :, None].astype(np.float32)
    return w.reshape(out_f, in_f)  # float32 (caller casts)


def _build_program(act=AF.Gelu):
    nc = bacc.Bacc(
        "TRN2",
        target_bir_lowering=False,
        debug=False,
        num_devices=1,
    )
    x_d = nc.dram_tensor("x", (NT, H), BF16, kind="ExternalInput").ap()
    hid_d = nc.dram_tensor("hid", (L, NT, H), BF16, kind="ExternalInput").ap()
    w1t_d = nc.dram_tensor("w1t", (H3, H), BF16, kind="ExternalInput").ap()
    w2t_d = nc.dram_tensor("w2t", (H, H), BF16, kind="ExternalInput").ap()
    b1s_d = nc.dram_tensor("b1s", (P, MO), F32, kind="ExternalInput").ap()
    b2b_d = nc.dram_tensor("b2b", (P, H), BF16, kind="ExternalInput").ap()
    g1b_d = nc.dram_tensor("g1b", (P, H), BF16, kind="ExternalInput").ap()
    g2b_d = nc.dram_tensor("g2b", (P, H), BF16, kind="ExternalInput").ap()
    bcb_d = nc.dram_tensor("bcb", (P, H), BF16, kind="ExternalInput").ap()
    out_d = nc.dram_tensor("out", (NT, H), BF16, kind="ExternalOutput").ap()

    with tile.TileContext(nc) as tc:
        _program(nc, tc, x_d, hid_d, w1t_d, w2t_d, b1s_d, b2b_d,
                 g1b_d, g2b_d, bcb_d, out_d, act)

    nc.compile()
    return nc


def _program(nc, tc, x_d, hid_d, w1t_d, w2t_d, b1s_d, b2b_d, g1b_d, g2b_d,
             bcb_d, out_d, act=AF.Gelu):
    with (
        tc.tile_pool(name="consts", bufs=1) as cpool,
        tc.tile_pool(name="hl", bufs=8) as hpool,
        tc.tile_pool(name="acc", bufs=3) as apool,
        tc.tile_pool(name="cat", bufs=2) as catpool,
        tc.tile_pool(name="gt", bufs=2) as gpool,
        tc.tile_pool(name="xn", bufs=2) as xpool,
        tc.tile_pool(name="fg", bufs=3) as fgpool,
        tc.tile_pool(name="outp", bufs=2) as opool,
        tc.tile_pool(name="stats", bufs=3) as spool,
        tc.tile_pool(name="tmp", bufs=2) as tpool,
        tc.tile_pool(name="psA", bufs=2, space="PSUM") as ps1pool,
        tc.tile_pool(name="psB", bufs=2, space="PSUM") as ps2pool,
        tc.tile_pool(name="psT", bufs=3, space="PSUM") as tpspool,
    ):
        # ---- first GEMM1 weight chunk + small consts on the scalar HWDGE
        # queue up front; the remaining weight chunks are interleaved into
        # the sync stream at the positions they are first needed, so the
        # supertile-0 hidden stream keeps (most of) the full DMA bandwidth
        # and compute ramps early instead of backloading.
        w1t_sb = cpool.tile([P, KO1, H], BF16, name="w1t")
        w1t_r = w1t_d.rearrange("(ko p) n -> p ko n", p=P)
        nc.scalar.dma_start(w1t_sb[:, 0:9, :], w1t_r[:, 0:9, :])
        w2t_sb = cpool.tile([P, KO2, H], BF16, name="w2t")
        b1_sb = cpool.tile([P, MO], F32, name="b1s")
        nc.scalar.dma_start(b1_sb, b1s_d)
        b2b_sb = cpool.tile([P, H], BF16, name="b2b")
        nc.scalar.dma_start(b2b_sb, b2b_d)
        g1b_sb = cpool.tile([P, H], BF16, name="g1b")
        nc.scalar.dma_start(g1b_sb, g1b_d)
        g2b_sb = cpool.tile([P, H], BF16, name="g2b")
        nc.scalar.dma_start(g2b_sb, g2b_d)
        bcb_sb = cpool.tile([P, H], BF16, name="bcb")
        nc.scalar.dma_start(bcb_sb, bcb_d)
        ident = cpool.tile([P, P], BF16, name="ident")
        masks.make_identity(nc, ident)

        st = [dict() for _ in SUPS]

        def pe_transpose(catT, tt, ko0, src):
            """catT[:, tt, ko0+k, :] = src[:, k*P:(k+1)*P].T for k in 0..MO.

            PE identity-transpose into PSUM, ACT copies back to SBUF. Keeps
            the HWDGE queues free of xbar transposes (dma_start_transpose
            drains its queue as deadlock protection, head-blocking the
            hidden-layer stream).
            """
            for g in range(3):
                tps = tpspool.tile([P, 3, P], BF16, tag="tps")
                for c in range(3):
                    k = g * 3 + c
                    nc.tensor.transpose(tps[:, c, :],
                                        src[:, k * P:(k + 1) * P], ident)
                nc.scalar.copy(catT[:, tt, ko0 + 3 * g:ko0 + 3 * (g + 1), :],
                               tps)

        def load_token_tile(dst, src2d, t0, ntok):
            """Load [ntok, H] DRAM rows into dst [P, 2, H] token-major."""
            nc.sync.dma_start(
                dst, src2d[t0:t0 + 2 * P, :].rearrange(
                    "(s p) f -> p s f", p=P))

        def accum_half(si, half, insert_s1T=False, inline=None):
            """Load 13 layers, DVE-chain them into an acc tile.

            ``inline`` maps layer-offset -> callable emitted right after that
            layer's load (used to splice weight-chunk DMAs into the sync
            stream at the position they are first needed).
            """
            t0, ntok = SUPS[si]
            l0 = 13 * half
            acc = apool.tile([P, NSUB, H], BF16, name=f"s{si}_{half}",
                             tag="acc")
            lts = []
            for i in range(2):
                lt = hpool.tile([P, NSUB, H], BF16, name=f"hl{si}_{l0+i}",
                                tag="hl")
                load_token_tile(lt, hid_d[l0 + i], t0, ntok)
                if inline and i in inline:
                    inline[i]()
                lts.append(lt)
            nc.vector.tensor_add(acc, lts[0], lts[1])
            for i in range(2, 13):
                lt = hpool.tile([P, NSUB, H], BF16, name=f"hl{si}_{l0+i}",
                                tag="hl")
                load_token_tile(lt, hid_d[l0 + i], t0, ntok)
                if inline and i in inline:
                    inline[i]()
                nc.vector.tensor_add(acc, acc, lt)
                if insert_s1T and i == 3:
                    # s1 is long done by now; transpose it mid-stream so
                    # GEMM1's k=9..17 can run before s2 lands.
                    S = st[si]
                    for tt in range(NSUB):
                        pe_transpose(S["catT"], tt, MO, S["s1"][:, tt, :])
            return acc

        def loads_a(si):
            t0, ntok = SUPS[si]
            S = st[si]
            x_nat = xpool.tile([P, NSUB, H], BF16, name=f"x{si}", tag="xnat")
            S["x"] = x_nat
            load_token_tile(x_nat, x_d, t0, ntok)
            catT = catpool.tile([P, NSUB, KO1, P], BF16, name=f"cat{si}",
                                tag="catT")
            S["catT"] = catT
            for tt in range(NSUB):
                pe_transpose(catT, tt, 0, x_nat[:, tt, :])
            agg = spool.tile([P, NSUB, 4], F32, name=f"agg{si}", tag="agg")
            S["agg"] = agg
            for tt in range(NSUB):
                bnx = spool.tile([P, 3, 6], F32, name=f"bnx{si}_{tt}",
                                 tag="bnx")
                for c in range(NCHUNK):
                    nc.vector.bn_stats(bnx[:, c, :],
                                       x_nat[:, tt, c * CH:(c + 1) * CH])
                nc.vector.bn_aggr(agg[:, tt, 0:2], bnx)
            inline = None
            if si == 0:
                inline = {6: lambda: nc.sync.dma_start(
                    w1t_sb[:, 9:18, :], w1t_r[:, 9:18, :])}
            S["s1"] = accum_half(si, 0, inline=inline)

        def loads_b(si):
            S = st[si]
            inline = None
            if si == 0:
                inline = {
                    2: lambda: nc.sync.dma_start(
                        w1t_sb[:, 18:27, :], w1t_r[:, 18:27, :]),
                    7: lambda: nc.sync.dma_start(
                        w2t_sb, w2t_d.rearrange("(ko p) n -> p ko n", p=P)),
                }
            S["s2"] = accum_half(si, 1, insert_s1T=True, inline=inline)
            for tt in range(NSUB):
                pe_transpose(S["catT"], tt, 2 * MO, S["s2"][:, tt, :])

        def tail_pe(si):
            S = st[si]
            catT = S["catT"]
            gT = gpool.tile([P, MO, NSUB * P], BF16, name=f"gT{si}", tag="gT")
            S["gT"] = gT
            for mm in range(MO):
                ps1 = ps1pool.tile([P, NSUB * P], F32, tag="ps1")
                for kk in range(KO1):
                    nc.tensor.matmul(
                        ps1.rearrange("p (a b) -> p a b", a=NSUB),
                        lhsT=w1t_sb[:, kk, mm * P:(mm + 1) * P],
                        rhs=catT[:, :, kk, :],
                        start=(kk == 0),
                        stop=(kk == KO1 - 1),
                    )
                nc.scalar.activation(gT[:, mm, :], ps1, act,
                                     bias=b1_sb[:, mm:mm + 1])
            fgs = []
            for tt in range(NSUB):
                fg = fgpool.tile([P, H], BF16, name=f"fg{si}_{tt}", tag="fg")
                fgs.append(fg)
                for nn in range(NCHUNK):
                    ps2 = ps2pool.tile([P, CH], F32, tag="ps2")
                    for kk in range(KO2):
                        nc.tensor.matmul(
                            ps2,
                            lhsT=gT[:, kk, tt * P:(tt + 1) * P],
                            rhs=w2t_sb[:, kk, nn * CH:(nn + 1) * CH],
                            start=(kk == 0),
                            stop=(kk == KO2 - 1),
                        )
                    nc.vector.tensor_tensor(
                        fg[:, nn * CH:(nn + 1) * CH], ps2,
                        b2b_sb[:, nn * CH:(nn + 1) * CH], ALU.add)
            S["fgs"] = fgs

        def tail_dve(si):
            t0, ntok = SUPS[si]
            S = st[si]
            agg = S["agg"]
            rpack = spool.tile([P, 2 * NSUB], F32, name=f"rp{si}", tag="rpack")
            for tt in range(NSUB):
                fg = S["fgs"][tt]
                bnf = spool.tile([P, 3, 6], F32, name=f"bnf{si}_{tt}",
                                 tag="bnf")
                for c in range(NCHUNK):
                    nc.vector.bn_stats(bnf[:, c, :],
                                       fg[:, c * CH:(c + 1) * CH])
                nc.vector.bn_aggr(agg[:, tt, 2:4], bnf)
                nc.vector.tensor_scalar_add(rpack[:, 2 * tt:2 * tt + 1],
                                            agg[:, tt, 1:2], EPS)
                nc.vector.tensor_scalar_add(rpack[:, 2 * tt + 1:2 * tt + 2],
                                            agg[:, tt, 3:4], EPS)
            ig = spool.tile([P, 2 * NSUB], F32, name=f"ig{si}", tag="ig")
            nc.vector.reciprocal(ig, rpack)
            nc.scalar.activation(ig, ig, AF.Sqrt)
            for tt in range(NSUB):
                fg = S["fgs"][tt]
                tmp1 = tpool.tile([P, H], BF16, tag="tmp1")
                tmp2 = tpool.tile([P, H], BF16, tag="tmp2")
                # tmp2 = (fg - mu2) * G2;  G2 = w*ln2_g broadcast
                nc.vector.scalar_tensor_tensor(
                    tmp2, fg, agg[:, tt, 2:3], g2b_sb,
                    ALU.subtract, ALU.mult)
                # tmp1 = (x - mu1) * G1;  G1 = (1-w)*ln1_g
                nc.vector.scalar_tensor_tensor(
                    tmp1, S["x"][:, tt, :], agg[:, tt, 0:1], g1b_sb,
                    ALU.subtract, ALU.mult)
                # tmp1 = tmp1 * ig1 + Bc;  Bc = w*ln2_b + (1-w)*ln1_b
                nc.vector.scalar_tensor_tensor(
                    tmp1, tmp1, ig[:, 2 * tt:2 * tt + 1], bcb_sb,
                    ALU.mult, ALU.add)
                out_t = opool.tile([P, H], BF16, name=f"o{si}_{tt}",
                                   tag="outt")
                nc.vector.scalar_tensor_tensor(
                    out_t, tmp2, ig[:, 2 * tt + 1:2 * tt + 2], tmp1,
                    ALU.mult, ALU.add)
                nc.scalar.dma_start(
                    out_d[t0 + tt * P:t0 + (tt + 1) * P, :], out_t)

        for si in range(len(SUPS)):
            loads_a(si)
            if si > 0:
                tail_dve(si - 1)
            loads_b(si)
            tail_pe(si)
        tail_dve(len(SUPS) - 1)


_NC_CACHE = {}


def _get_nc():
    if "nc" not in _NC_CACHE:
        _NC_CACHE["nc"] = _build_program()
    return _NC_CACHE["nc"]


def _host_prep(codes1, scales1, b1, codes2, scales2, b2,
               ln1_g, ln1_b, ln2_g, ln2_b, alpha):
    # W1 with 1/13 folded into the f1/f2 column blocks (mean -> sum)
    w1 = _dequant_nf4(codes1, scales1)
    # match reference rounding: dequant result is cast to bf16 first
    w1 = w1.astype(NP_BF16).astype(np.float32)
    w1[:, H:] *= np.float32(1.0 / 13.0)
    w1t = np.ascontiguousarray(w1.T).astype(NP_BF16)

    w2 = _dequant_nf4(codes2, scales2).astype(NP_BF16)
    w2t = np.ascontiguousarray(w2.astype(np.float32).T).astype(NP_BF16)

    b1s = np.ascontiguousarray(
        b1.astype(np.float32).reshape(MO, P).T)  # [P, MO]

    b2b = np.ascontiguousarray(
        np.broadcast_to(b2.astype(NP_BF16), (P, H)))

    a32 = alpha.astype(np.float32)
    w_gate = (1.0 / (1.0 + np.exp(-a32[0]))).astype(NP_BF16)
    one_minus = (NP_BF16(1.0) - w_gate)
    g1 = (one_minus.astype(np.float32) * ln1_g.astype(np.float32))
    g2 = (w_gate.astype(np.float32) * ln2_g.astype(np.float32))
    bc = (w_gate.astype(np.float32) * ln2_b.astype(np.float32)
          + one_minus.astype(np.float32) * ln1_b.astype(np.float32))
    g1b = np.ascontiguousarray(np.broadcast_to(g1.astype(NP_BF16), (P, H)))
    g2b = np.ascontiguousarray(np.broadcast_to(g2.astype(NP_BF16), (P, H)))
    bcb = np.ascontiguousarray(np.broadcast_to(bc.astype(NP_BF16), (P, H)))
    return w1t, w2t, b1s, b2b, g1b, g2b, bcb


def make_in_maps(image_features, hidden, codes1, scales1, b1, codes2, scales2,
                 b2, ln1_g, ln1_b, ln2_g, ln2_b, alpha):
    w1t, w2t, b1s, b2b, g1b, g2b, bcb = _host_prep(
        codes1, scales1, b1, codes2, scales2, b2,
        ln1_g, ln1_b, ln2_g, ln2_b, alpha)
    B = image_features.shape[0]
    in_maps = []
    for c in range(B):
        in_maps.append({
            "x": np.ascontiguousarray(image_features[c]).astype(NP_BF16, copy=False),
            "hid": np.ascontiguousarray(hidden[:, c]).astype(NP_BF16, copy=False),
            "w1t": w1t, "w2t": w2t, "b1s": b1s, "b2b": b2b,
            "g1b": g1b, "g2b": g2b, "bcb": bcb,
        })
    return in_maps


def kernel(image_features, hidden, codes1, scales1, b1, codes2, scales2, b2,
           ln1_g, ln1_b, ln2_g, ln2_b, alpha, _trace=False):
    B, N, Hin = image_features.shape
    assert (B, N, Hin) == (8, NT, H), (B, N, Hin)
    nc = _get_nc()
    in_maps = make_in_maps(image_features, hidden, codes1, scales1, b1,
                           codes2, scales2, b2, ln1_g, ln1_b, ln2_g, ln2_b,
                           alpha)
    res = bass_utils.run_bass_kernel_spmd(
        nc, in_maps, core_ids=list(range(8)), trace=_trace)
    out = np.stack([res.results[c]["out"] for c in range(8)])
    if _trace:
        kernel._last_results = res
    return out.astype(image_features.dtype, copy=False)
